# revision 12
# baseline (speedup 1.0000x reference)
"""DPCL objective (deep-clustering loss) on 8 Trainium2 NeuronCores.

Sharding: pure data parallel -- batch dim N=16 -> 2 utterances per core.
For each utterance the loss reduces to the 42x40 weighted Gram matrix

    G = [W*E | wo0 | wo1]^T @ E        (contraction over FT = 154200)

where E is the (FT, 40) embedding, W = diag(magnitude_mix row), and
wo_s = magnitude_mix * onehot_s(argmax(magnitude_ref)).  Because the
weights enter the affinity bilinearly, scaling ONE matmul operand by the
un-normalized magnitudes is enough (no sqrt, no normalization on device):

    A  = out^T out = G[0:40] / M,   C^T = G[40:42] / M,
    B  = diag(b0, b1)/M,  b_s = sum_k wo_s[k],  M = b0 + b1
    loss_n = (||A||^2 + ||B||^2 - 2||C||^2) / (M^2 T)

Host work is limited to slicing inputs per core, casting the embedding to
bf16 (PSUM accumulation stays fp32; end-to-end rel err ~6e-5), and
combining the 16 tiny Gram matrices into the scalar loss.

Device kernel (default "tile2" build), per core:
  - FT layout 154200 = 128*1204 + 88: partition p owns rows
    [p*1204, (p+1)*1204), so every DMA is per-partition contiguous with
    zero host-side copies; the 88-row tail is one extra small-K matmul.
  - PE: 2410 accumulating 128x42x40 matmuls, alternating between two
    column-tile positions (0,0)/(0,64) of the 128x128 array so pairs of
    chunks run concurrently (two 42-col stationaries side by side); the
    two partial Grams are summed on the host.
  - DVE: the weighted copy W*E.  The d-broadcast weights are first
    materialized in (c,d) bf16 layout by the scalar engine ("wrep"),
    which makes the multiply an all-bf16 stride-1 op -> DVE packed 2x
    mode (1.95us vs 3.73us per 86-chunk group).
  - ACT: wrep builds, wo-column copies into the stationary, PSUM->SBUF.
  - DMA: E tiles spread over three independent queue rows (SWDGE q0 via
    GpSimd fire-and-forget, SP HWDGE q1, ACT HWDGE q10) -- a single HWDGE
    ring blocks its issuing engine for the whole transfer.
Measured ~145-165 us HW time (8 cores, ~28.7 MB HBM reads/core).
"""

import os
import sys
import numpy as np
from contextlib import ExitStack

sys.path.insert(0, "/opt/trn_rl_repo")

N_FULL = 16
F, T, S, D = 257, 600, 2, 40
FT = F * T                      # 154200
NCORES = 8
NPER = N_FULL // NCORES         # 2 utterances per core
P = 128

# full-size FT decomposition: FT = P*CPP + TAIL
CPP = FT // P                   # 1204 columns per partition (main part)
MAIN = P * CPP                  # 154112
TAIL = FT - MAIN                # 88
CB = 86                         # chunks per group
NGROUPS = CPP // CB             # 14

# matmul operand dtype / transfer strategy:
#   "f32"      - fp32 matmuls (4 cyc/row), fp32 DMA
#   "bf16"     - bf16 matmuls, cast-during-DMA (SWDGE), fp32 HBM reads
#   "bf16host" - bf16 matmuls, embedding pre-cast on host (halves HBM reads)
#   "perm"     - bf16host + host-permuted [P, D, c] layout (packed 2x DVE
#                weighted-copy) + PE column-tiling (2 concurrent chunks)
MODE = os.environ.get("DPCL_MODE", "v2")
EW = int(os.environ.get("DPCL_EW", "172"))  # E-tile chunk width
EBUFS = int(os.environ.get("DPCL_EBUFS", "5"))
PBUFS = int(os.environ.get("DPCL_PBUFS", "2"))
NG_POOL = int(os.environ.get("DPCL_NGPOOL", "0"))      # WE groups on GpSimd
PREP_POOL = os.environ.get("DPCL_PREPPOOL", "0") == "1"  # mask prep on GpSimd
ERINGS = int(os.environ.get("DPCL_ERINGS", "3"))
BDVE = int(os.environ.get("DPCL_BDVE", "2"))  # of each 7 wrep builds, this many on DVE
K7 = int(os.environ.get("DPCL_K7", "7"))  # of each 7 tiles, this many use ACT-wrep
# rank of each position in the 7-cycle: positions with rank < K7 use wrep.
WREP_PAT = (
    [0, 1, 5, 2, 3, 6, 4]
    if os.environ.get("DPCL_PAT", "id") == "il"
    else [0, 1, 2, 3, 4, 5, 6]
)

LAST_EXEC_NS = None

_prog_cache = {}


def _build_program(nper, cpp, cb, ngroups, tail, mode):
    import concourse.bass as bass
    import concourse.bacc as bacc
    import concourse.tile as tile
    from concourse import mybir

    f32 = mybir.dt.float32
    dmm = f32 if mode == "f32" else mybir.dt.bfloat16
    ft = P * cpp + tail
    main = P * cpp
    assert ngroups * cb == cpp

    nc = bacc.Bacc(
        "TRN2", target_bir_lowering=False, debug=False, num_devices=NCORES
    )
    emb_dt = dmm if mode == "bf16host" else f32
    emb = nc.declare_dram_parameter("emb", [nper, ft, D], emb_dt, isOutput=False)
    mm = nc.declare_dram_parameter("mm", [nper, ft], f32, isOutput=False)
    mref = nc.declare_dram_parameter("mref", [nper, ft, S], f32, isOutput=False)
    g_out = nc.declare_dram_parameter("g_out", [nper, D + S, D], f32, isOutput=True)
    b_out = nc.declare_dram_parameter("b_out", [nper, P, S], f32, isOutput=True)

    # engine used for the big E loads (SWDGE supports dtype-cast during DMA)
    if mode == "bf16":
        e_dma = lambda out, in_: nc.gpsimd.dma_start(out=out, in_=in_)
    else:
        e_dma = lambda out, in_: nc.sync.dma_start(out=out, in_=in_)
    # in bf16 (cast-DMA) mode GpSimd is busy generating descriptors; otherwise
    # split the big weighted-copy work between DVE and GpSimd
    split_we = mode != "bf16"

    with tile.TileContext(nc) as tc, ExitStack() as ctx:
        wpool = ctx.enter_context(tc.tile_pool(name="wpool", bufs=2))
        epool = ctx.enter_context(tc.tile_pool(name="epool", bufs=3))
        lpool = ctx.enter_context(tc.tile_pool(name="lpool", bufs=3))
        spool = ctx.enter_context(tc.tile_pool(name="spool", bufs=2))
        psum = ctx.enter_context(tc.tile_pool(name="psum", bufs=2, space="PSUM"))

        for u in range(nper):
            # ---- per-row weight / mask prep (all [128, cpp]) ----
            w_t = wpool.tile([P, cpp], f32, tag="w")
            nc.sync.dma_start(
                out=w_t[:], in_=mm[u, 0:main].rearrange("(p c) -> p c", p=P)
            )
            mr_t = wpool.tile([P, cpp * S], f32, tag="mr")
            nc.sync.dma_start(
                out=mr_t[:],
                in_=mref[u, 0:main, :].rearrange("(p c) s -> p (c s)", p=P),
            )
            mr3 = mr_t[:].rearrange("p (c s) -> p c s", s=S)
            mask_t = wpool.tile([P, cpp], f32, tag="mask")
            # mask = 1.0 where speaker-1 magnitude wins the argmax
            nc.vector.tensor_tensor(
                mask_t[:], mr3[:, :, 1], mr3[:, :, 0], mybir.AluOpType.is_gt
            )
            wo_t = wpool.tile([P, S * cpp], f32, tag="wo")  # [wo0 | wo1]
            nc.vector.tensor_mul(wo_t[:, cpp : 2 * cpp], w_t[:], mask_t[:])
            nc.vector.tensor_sub(wo_t[:, 0:cpp], w_t[:], wo_t[:, cpp : 2 * cpp])
            wo3 = wo_t[:].rearrange("p (s c) -> p c s", s=S)

            wored = spool.tile([P, S], f32, tag="wored")
            nc.vector.tensor_reduce(
                wored[:],
                wo_t[:].rearrange("p (s c) -> p s c", s=S),
                mybir.AxisListType.X,
                mybir.AluOpType.add,
            )

            # ---- tail prep ([tail, *]) ----
            wtl = spool.tile([P, 1], f32, tag="wtl")
            nc.sync.dma_start(out=wtl[0:tail, :], in_=mm[u, main:ft].unsqueeze(1))
            mrtl = spool.tile([P, S], f32, tag="mrtl")
            nc.sync.dma_start(out=mrtl[0:tail, :], in_=mref[u, main:ft, :])
            masktl = spool.tile([P, 1], f32, tag="masktl")
            nc.vector.tensor_tensor(
                masktl[0:tail, :],
                mrtl[0:tail, 1:2],
                mrtl[0:tail, 0:1],
                mybir.AluOpType.is_gt,
            )
            wotl = spool.tile([P, S], f32, tag="wotl")
            nc.vector.tensor_mul(wotl[0:tail, 1:2], wtl[0:tail, :], masktl[0:tail, :])
            nc.vector.tensor_sub(wotl[0:tail, 0:1], wtl[0:tail, :], wotl[0:tail, 1:2])
            nc.vector.tensor_add(wored[0:tail, :], wored[0:tail, :], wotl[0:tail, :])
            nc.sync.dma_start(out=b_out[u, :, :], in_=wored[:])

            # ---- Gram accumulation ----
            gp = psum.tile([D + S, D], f32, tag="g")
            e_main = emb[u, 0:main, :].rearrange("(p c) d -> p c d", p=P)
            for g in range(ngroups):
                et = epool.tile([P, cb * D], dmm, tag="e")
                e3 = et[:].rearrange("p (c d) -> p c d", d=D)
                e_dma(e3[:], e_main[:, g * cb : (g + 1) * cb, :])

                lt = lpool.tile([P, cb * (D + S)], dmm, tag="l")
                l3 = lt[:].rearrange("p (c e) -> p c e", e=D + S)
                # weighted copy of E into the stationary operand
                wslice = w_t[:, g * cb : (g + 1) * cb].unsqueeze(2).broadcast_to(
                    [P, cb, D]
                )
                weng = nc.gpsimd if (split_we and g % 2 == 1) else nc.vector
                weng.tensor_mul(l3[:, :, 0:D], e3[:], wslice)
                # masked-weight columns (wo0, wo1)
                weng.tensor_copy(
                    l3[:, :, D : D + S], wo3[:, g * cb : (g + 1) * cb, :]
                )
                for c in range(cb):
                    nc.tensor.matmul(
                        gp[:],
                        lt[:, c * (D + S) : (c + 1) * (D + S)],
                        et[:, c * D : (c + 1) * D],
                        start=(g == 0 and c == 0),
                        stop=False,
                    )

            # tail chunk (contraction dim = tail)
            etl = spool.tile([P, D], dmm, tag="etl")
            e_dma(etl[0:tail, :], emb[u, main:ft, :])
            ltl = spool.tile([P, D + S], dmm, tag="ltl")
            nc.vector.tensor_mul(
                ltl[0:tail, 0:D],
                etl[0:tail, :],
                wtl[0:tail, :].broadcast_to([tail, D]),
            )
            nc.vector.tensor_copy(ltl[0:tail, D : D + S], wotl[0:tail, :])
            nc.tensor.matmul(
                gp[:], ltl[0:tail, :], etl[0:tail, :], start=False, stop=True
            )

            gsb = spool.tile([D + S, D], f32, tag="gsb")
            nc.scalar.activation(gsb[:], gp[:], mybir.ActivationFunctionType.Copy)
            nc.sync.dma_start(out=g_out[u, :, :], in_=gsb[:])

    nc.compile()
    return nc


def _build_perm(nper, cpp, ew, cb, tail):
    """Permuted-layout bf16 build: E arrives as [nper, P, D, cpp] so the
    weighted copy hits DVE's packed 2x mode, and chunks alternate between
    two PE column-tile positions (the 42-col stationary only uses a third
    of the array)."""
    import concourse.bacc as bacc
    import concourse.tile as tile
    from concourse import mybir

    f32 = mybir.dt.float32
    bf16 = mybir.dt.bfloat16
    ft = P * cpp + tail
    main = P * cpp
    ntiles = cpp // ew
    gpe = ew // cb
    assert ntiles * ew == cpp and gpe * cb == ew and cb % 2 == 0

    nc = bacc.Bacc(
        "TRN2", target_bir_lowering=False, debug=False, num_devices=NCORES
    )
    emb_p = nc.declare_dram_parameter("emb_p", [nper, P, D, cpp], bf16, isOutput=False)
    emb_t = nc.declare_dram_parameter("emb_t", [nper, tail, D], bf16, isOutput=False)
    mm = nc.declare_dram_parameter("mm", [nper, ft], f32, isOutput=False)
    mref = nc.declare_dram_parameter("mref", [nper, ft, S], f32, isOutput=False)
    g_out = nc.declare_dram_parameter(
        "g_out", [nper, 2, D + S, D], f32, isOutput=True
    )
    b_out = nc.declare_dram_parameter("b_out", [nper, P, S], f32, isOutput=True)

    with tile.TileContext(nc) as tc, ExitStack() as ctx:
        wpool = ctx.enter_context(tc.tile_pool(name="wpool", bufs=2))
        epool = ctx.enter_context(tc.tile_pool(name="epool", bufs=3))
        lpool = ctx.enter_context(tc.tile_pool(name="lpool", bufs=3))
        spool = ctx.enter_context(tc.tile_pool(name="spool", bufs=2))
        psum = ctx.enter_context(tc.tile_pool(name="psum", bufs=2, space="PSUM"))

        for u in range(nper):
            # ---- per-row weight / mask prep (all [128, cpp], fp32) ----
            w_t = wpool.tile([P, cpp], f32, tag="w")
            nc.sync.dma_start(
                out=w_t[:], in_=mm[u, 0:main].rearrange("(p c) -> p c", p=P)
            )
            mr_t = wpool.tile([P, cpp * S], f32, tag="mr")
            nc.sync.dma_start(
                out=mr_t[:],
                in_=mref[u, 0:main, :].rearrange("(p c) s -> p (c s)", p=P),
            )
            mr3 = mr_t[:].rearrange("p (c s) -> p c s", s=S)
            mask_t = wpool.tile([P, cpp], f32, tag="mask")
            nc.vector.tensor_tensor(
                mask_t[:], mr3[:, :, 1], mr3[:, :, 0], mybir.AluOpType.is_gt
            )
            wo_t = wpool.tile([P, S * cpp], f32, tag="wo")  # [wo0 | wo1]
            nc.vector.tensor_mul(wo_t[:, cpp : 2 * cpp], w_t[:], mask_t[:])
            nc.vector.tensor_sub(wo_t[:, 0:cpp], w_t[:], wo_t[:, cpp : 2 * cpp])
            wo_sc = wo_t[:].rearrange("p (s c) -> p s c", s=S)
            w_bf = wpool.tile([P, cpp], bf16, tag="wbf")
            nc.vector.tensor_copy(w_bf[:], w_t[:])

            wored = spool.tile([P, S], f32, tag="wored")
            nc.vector.tensor_reduce(
                wored[:],
                wo_t[:].rearrange("p (s c) -> p s c", s=S),
                mybir.AxisListType.X,
                mybir.AluOpType.add,
            )

            # ---- tail prep ----
            wtl = spool.tile([P, 1], f32, tag="wtl")
            nc.sync.dma_start(out=wtl[0:tail, :], in_=mm[u, main:ft].unsqueeze(1))
            mrtl = spool.tile([P, S], f32, tag="mrtl")
            nc.sync.dma_start(out=mrtl[0:tail, :], in_=mref[u, main:ft, :])
            masktl = spool.tile([P, 1], f32, tag="masktl")
            nc.vector.tensor_tensor(
                masktl[0:tail, :],
                mrtl[0:tail, 1:2],
                mrtl[0:tail, 0:1],
                mybir.AluOpType.is_gt,
            )
            wotl = spool.tile([P, S], f32, tag="wotl")
            nc.vector.tensor_mul(wotl[0:tail, 1:2], wtl[0:tail, :], masktl[0:tail, :])
            nc.vector.tensor_sub(wotl[0:tail, 0:1], wtl[0:tail, :], wotl[0:tail, 1:2])
            nc.vector.tensor_add(wored[0:tail, :], wored[0:tail, :], wotl[0:tail, :])
            nc.sync.dma_start(out=b_out[u, :, :], in_=wored[:])

            # ---- Gram accumulation, two column-tile positions ----
            gp = psum.tile([P, D], f32, tag="g")
            started = [False, False]
            for t in range(ntiles):
                et = epool.tile([P, D * ew], bf16, tag="e")
                e3 = et[:].rearrange("p (d c) -> p d c", c=ew)
                nc.sync.dma_start(
                    out=e3[:], in_=emb_p[u, :, :, t * ew : (t + 1) * ew]
                )
                for gc in range(gpe):
                    co = gc * cb
                    lt = lpool.tile([P, cb * (D + S)], bf16, tag="l")
                    l3 = lt[:].rearrange("p (e c) -> p e c", c=cb)
                    wsl = (
                        w_bf[:, t * ew + co : t * ew + co + cb]
                        .unsqueeze(1)
                        .broadcast_to([P, D, cb])
                    )
                    nc.vector.tensor_mul(l3[:, 0:D, :], e3[:, :, co : co + cb], wsl)
                    nc.vector.tensor_copy(
                        l3[:, D : D + S, :],
                        wo_sc[:, :, t * ew + co : t * ew + co + cb],
                    )
                    for c in range(cb):
                        k = t * ew + co + c
                        par = k % 2
                        pb = 64 * par
                        st = not started[par]
                        started[par] = True
                        nc.tensor.matmul(
                            gp[pb : pb + D + S, :],
                            l3[:, :, c : c + 1],
                            e3[:, :, co + c : co + c + 1],
                            start=st,
                            stop=(par == 1 and k == cpp - 1),
                            tile_position=(0, pb),
                            skip_group_check=True,
                        )

            # tail chunk -> position 0 accumulator, closes its group
            etl = spool.tile([P, D], bf16, tag="etl")
            nc.sync.dma_start(out=etl[0:tail, :], in_=emb_t[u, :, :])
            ltl = spool.tile([P, D + S], bf16, tag="ltl")
            nc.vector.tensor_mul(
                ltl[0:tail, 0:D],
                etl[0:tail, :],
                wtl[0:tail, :].broadcast_to([tail, D]),
            )
            nc.vector.tensor_copy(ltl[0:tail, D : D + S], wotl[0:tail, :])
            nc.tensor.matmul(
                gp[0 : D + S, :],
                ltl[0:tail, :],
                etl[0:tail, :],
                start=False,
                stop=True,
                tile_position=(0, 0),
                skip_group_check=True,
            )

            gsb = spool.tile([P, D], f32, tag="gsb")
            nc.scalar.activation(
                gsb[0 : D + S, :], gp[0 : D + S, :], mybir.ActivationFunctionType.Copy
            )
            nc.scalar.activation(
                gsb[64 : 64 + D + S, :],
                gp[64 : 64 + D + S, :],
                mybir.ActivationFunctionType.Copy,
            )
            nc.sync.dma_start(out=g_out[u, 0, :, :], in_=gsb[0 : D + S, :])
            nc.sync.dma_start(out=g_out[u, 1, :, :], in_=gsb[64 : 64 + D + S, :])

    nc.compile()
    return nc


def _build_tile2(nper, cpp, ew, cb, tail, ng_pool=0, prep_pool=True):
    """Contiguous (c,d) layouts for all PE operands + 2-way PE column
    tiling + DVE/GpSimd split of the weighted copy + dual HWDGE rings."""
    import concourse.bacc as bacc
    import concourse.tile as tile
    from concourse import mybir

    f32 = mybir.dt.float32
    bf16 = mybir.dt.bfloat16
    ft = P * cpp + tail
    main = P * cpp
    ntiles = cpp // ew
    gpe = ew // cb
    assert ntiles * ew == cpp and gpe * cb == cb * gpe and gpe * cb == ew

    nc = bacc.Bacc(
        "TRN2", target_bir_lowering=False, debug=False, num_devices=NCORES
    )
    emb = nc.declare_dram_parameter("emb", [nper, ft, D], bf16, isOutput=False)
    f16 = mybir.dt.float16
    mm = nc.declare_dram_parameter("mm", [nper, ft], f16, isOutput=False)
    mref = nc.declare_dram_parameter("mref", [nper, ft, S], f16, isOutput=False)
    g_out = nc.declare_dram_parameter(
        "g_out", [nper, 2, D + S, D], f32, isOutput=True
    )
    b_out = nc.declare_dram_parameter("b_out", [nper, P, S], f32, isOutput=True)

    total_groups = nper * ntiles * gpe

    with tile.TileContext(nc) as tc, ExitStack() as ctx:
        wpool = ctx.enter_context(tc.tile_pool(name="wpool", bufs=2))
        ppool = ctx.enter_context(tc.tile_pool(name="ppool", bufs=PBUFS))
        epool = ctx.enter_context(tc.tile_pool(name="epool", bufs=EBUFS))
        lpool = ctx.enter_context(tc.tile_pool(name="lpool", bufs=3))
        wrpool = ctx.enter_context(tc.tile_pool(name="wrpool", bufs=2))
        spool = ctx.enter_context(tc.tile_pool(name="spool", bufs=2))
        psum = ctx.enter_context(tc.tile_pool(name="psum", bufs=2, space="PSUM"))

        gi = 0  # global group index for the DVE/GpSimd split
        prep = {}
        for u in range(nper):
            # ---- per-row weight / mask prep (fp32 [128, cpp]) ----
            w_t = wpool.tile([P, cpp], f16, tag="w")
            nc.sync.dma_start(
                out=w_t[:], in_=mm[u, 0:main].rearrange("(p c) -> p c", p=P)
            )
            mr_t = ppool.tile([P, cpp * S], f16, tag="mr")
            nc.sync.dma_start(
                out=mr_t[:],
                in_=mref[u, 0:main, :].rearrange("(p c) s -> p (c s)", p=P),
            )
            mr3 = mr_t[:].rearrange("p (c s) -> p c s", s=S)
            peng = nc.gpsimd if prep_pool else nc.vector
            mask_t = ppool.tile([P, cpp], f32, tag="mask")
            nc.vector.tensor_tensor(
                mask_t[:], mr3[:, :, 1], mr3[:, :, 0], mybir.AluOpType.is_gt
            )
            wo_t = wpool.tile([P, S * cpp], f32, tag="wo")  # [wo0 | wo1]
            peng.tensor_mul(wo_t[:, cpp : 2 * cpp], w_t[:], mask_t[:])
            peng.tensor_sub(wo_t[:, 0:cpp], w_t[:], wo_t[:, cpp : 2 * cpp])
            wo_sc = wo_t[:].rearrange("p (s c) -> p s c", s=S)

            wored = spool.tile([P, S], f32, tag="wored")
            nc.vector.tensor_reduce(
                wored[:],
                wo_t[:].rearrange("p (s c) -> p s c", s=S),
                mybir.AxisListType.X,
                mybir.AluOpType.add,
            )

            # ---- tail prep ----
            wtl = spool.tile([P, 1], f16, tag="wtl")
            nc.sync.dma_start(out=wtl[0:tail, :], in_=mm[u, main:ft].unsqueeze(1))
            mrtl = spool.tile([P, S], f16, tag="mrtl")
            nc.sync.dma_start(out=mrtl[0:tail, :], in_=mref[u, main:ft, :])
            masktl = spool.tile([P, 1], f32, tag="masktl")
            nc.vector.tensor_tensor(
                masktl[0:tail, :],
                mrtl[0:tail, 1:2],
                mrtl[0:tail, 0:1],
                mybir.AluOpType.is_gt,
            )
            wotl = spool.tile([P, S], f32, tag="wotl")
            nc.vector.tensor_mul(wotl[0:tail, 1:2], wtl[0:tail, :], masktl[0:tail, :])
            nc.vector.tensor_sub(wotl[0:tail, 0:1], wtl[0:tail, :], wotl[0:tail, 1:2])
            nc.vector.tensor_add(wored[0:tail, :], wored[0:tail, :], wotl[0:tail, :])
            nc.sync.dma_start(out=b_out[u, :, :], in_=wored[:])
            prep[u] = (w_t, wo_sc, wtl, wotl)

        for u in range(nper):
            w_t, wo_sc, wtl, wotl = prep[u]
            # ---- Gram accumulation ----
            gp = psum.tile([P, D], f32, tag="g")
            started = [False, False]
            e_main = emb[u, 0:main, :].rearrange("(p c) d -> p c d", p=P)
            for t in range(ntiles):
                et = epool.tile([P, ew * D], bf16, tag="e")
                e3 = et[:].rearrange("p (c d) -> p c d", d=D)
                # spread the big loads over three independent DMA queue rows:
                # SWDGE (q0, fire-and-forget after ~1us Q7 emission), the SP
                # HWDGE ring (q1) and the ACT HWDGE ring (q10)
                if ERINGS == 2:
                    ering = (nc.gpsimd, nc.sync)[t % 2]
                else:
                    ering = (nc.sync, nc.gpsimd, nc.scalar)[t % 3]
                ering.dma_start(out=e3[:], in_=e_main[:, t * ew : (t + 1) * ew, :])

                lt = lpool.tile([P, ew * (D + S)], bf16, tag="l")
                l3 = lt[:].rearrange("p (c e) -> p c e", e=D + S)
                wsl = (
                    w_t[:, t * ew : (t + 1) * ew]
                    .unsqueeze(2)
                    .broadcast_to([P, ew, D])
                )
                # For most tiles, materialize the d-broadcast weights in (c,d)
                # bf16 layout on the otherwise-idle ACT engine; the weighted
                # copy then runs all-bf16 step-1 => DVE packed 2x mode
                # (1.95us vs 3.73us per group).  The rest run the direct 1x
                # broadcast multiply on DVE, balancing ACT vs DVE.
                use_wrep = WREP_PAT[gi % 7] < K7
                if use_wrep:
                    wrt = wrpool.tile([P, ew * D], bf16, tag="wr")
                    wr3 = wrt[:].rearrange("p (c d) -> p c d", d=D)
                    if WREP_PAT[gi % 7] >= 7 - BDVE:
                        nc.vector.tensor_copy(wr3[:], wsl)
                    else:
                        nc.scalar.activation(
                            wr3[:], wsl, mybir.ActivationFunctionType.Copy
                        )
                # one wo-columns copy per tile (ACT, overhead-dominated)
                nc.vector.tensor_copy(
                    l3[:, :, D : D + S],
                    wo_sc[:, :, t * ew : (t + 1) * ew].transpose([0, 2, 1]),
                )
                nc.vector.tensor_mul(
                    l3[:, :, 0:D], e3[:], wr3[:] if use_wrep else wsl
                )
                for gc in range(gpe):
                    co = gc * cb
                    gi += 1
                    for c in range(cb):
                        k = t * ew + co + c
                        par = k % 2
                        pb = 64 * par
                        st = not started[par]
                        started[par] = True
                        nc.tensor.matmul(
                            gp[pb : pb + D + S, :],
                            lt[:, (co + c) * (D + S) : (co + c + 1) * (D + S)],
                            et[:, (co + c) * D : (co + c + 1) * D],
                            start=st,
                            stop=(par == 1 and k == cpp - 1),
                            tile_position=(0, pb),
                            skip_group_check=True,
                        )

            # tail chunk -> position 0 accumulator, closes its group
            etl = spool.tile([P, D], bf16, tag="etl")
            nc.sync.dma_start(out=etl[0:tail, :], in_=emb[u, main:ft, :])
            ltl = spool.tile([P, D + S], bf16, tag="ltl")
            nc.vector.tensor_mul(
                ltl[0:tail, 0:D],
                etl[0:tail, :],
                wtl[0:tail, :].broadcast_to([tail, D]),
            )
            nc.vector.tensor_copy(ltl[0:tail, D : D + S], wotl[0:tail, :])
            nc.tensor.matmul(
                gp[0 : D + S, :],
                ltl[0:tail, :],
                etl[0:tail, :],
                start=False,
                stop=True,
                tile_position=(0, 0),
                skip_group_check=True,
            )

            gsb = spool.tile([P, D], f32, tag="gsb")
            nc.scalar.activation(
                gsb[0 : D + S, :], gp[0 : D + S, :], mybir.ActivationFunctionType.Copy
            )
            nc.scalar.activation(
                gsb[64 : 64 + D + S, :],
                gp[64 : 64 + D + S, :],
                mybir.ActivationFunctionType.Copy,
            )
            nc.sync.dma_start(out=g_out[u, 0, :, :], in_=gsb[0 : D + S, :])
            nc.sync.dma_start(out=g_out[u, 1, :, :], in_=gsb[64 : 64 + D + S, :])

    nc.compile()
    return nc


EW2 = int(os.environ.get("DPCL_EW2", "240"))       # chunks per full tile (mult of 3)
OMEGA = int(os.environ.get("DPCL_OMEGA", "8"))     # wrep width (divides D)
# per-full-tile class chars, tiles in order (u0 t0..t4, u1 t0..t4):
#   c = SWDGE cast fp8->bf16 E + DVE mul
#   p = plain bf16 E (SP/ACT HWDGE) + DVE mul
#   g = raw fp8 E + GPSIMD mul (mixed-dtype matmul moving operand)
#   G = plain bf16 E + GPSIMD mul
PAT2 = os.environ.get("DPCL_PAT2", "ccpccccpcc")
WRENG = os.environ.get("DPCL_WRENG", "act")        # wrep engine: act|vec
ACC = os.environ.get("DPCL_ACC", "1") == "1"       # ride sums on accum_out
EBUFS2 = int(os.environ.get("DPCL_EBUFS2", "5"))
LBUFS2 = int(os.environ.get("DPCL_LBUFS2", "2"))


def _build_v2(nper, cpp, ew, tail, pat):
    """FWL-window Gram build with fully tiled prep.

    One 128-col LDWEIGHTS window per 3-chunk block (overlapping windows over
    the contiguous (c,e) L layout trigger Fast Weight Load), one N=120 matmul
    per block accumulating a block-diagonal [128,120] PSUM whose three 42x40
    diagonal blocks are summed on the host.  L columns per chunk:
    [w*E (40) | wo1 | w]; C0/b0 are recovered on the host as t - C1 / M - b1.
    All prep (w / mref-plane loads, argmax mask, wo1) happens in tile-sized
    slices inside the pipeline so there is no serial prologue."""
    import concourse.bacc as bacc
    import concourse.tile as tile
    from concourse import mybir

    f32 = mybir.dt.float32
    bf16 = mybir.dt.bfloat16
    fp8 = mybir.dt.float8e4
    ft = P * cpp + tail
    main = P * cpp
    D1 = D + S                       # 42 stationary cols per chunk
    NFULL_T = cpp // ew              # full tiles per utterance
    REM = cpp - NFULL_T * ew         # leftover chunks
    NTT = NFULL_T + (1 if REM else 0)
    assert ew % 3 == 0 and D % OMEGA == 0
    assert len(pat) == nper * NFULL_T, (pat, NFULL_T)

    nc = bacc.Bacc(
        "TRN2", target_bir_lowering=False, debug=False, num_devices=NCORES
    )
    emb8 = nc.declare_dram_parameter("emb8", [nper, ft, D], fp8, isOutput=False)
    emb16 = nc.declare_dram_parameter("emb16", [nper, ft, D], bf16, isOutput=False)
    mm = nc.declare_dram_parameter("mm", [nper, ft], bf16, isOutput=False)
    mref0 = nc.declare_dram_parameter("mref0", [nper, ft], bf16, isOutput=False)
    mref1 = nc.declare_dram_parameter("mref1", [nper, ft], bf16, isOutput=False)
    g_out = nc.declare_dram_parameter("g_out", [nper, P, 120], f32, isOutput=True)
    b_out = nc.declare_dram_parameter("b_out", [nper, P, S], f32, isOutput=True)

    with tile.TileContext(nc) as tc, ExitStack() as ctx:
        wpool = ctx.enter_context(tc.tile_pool(name="wpool", bufs=3))
        epool = ctx.enter_context(tc.tile_pool(name="epool", bufs=EBUFS2))
        lpool = ctx.enter_context(tc.tile_pool(name="lpool", bufs=LBUFS2))
        wrpool = ctx.enter_context(tc.tile_pool(name="wrpool", bufs=2))
        spool = ctx.enter_context(tc.tile_pool(name="spool", bufs=2))
        psum = ctx.enter_context(tc.tile_pool(name="psum", bufs=2, space="PSUM"))

        for u in range(nper):
            upat = pat[u * NFULL_T : (u + 1) * NFULL_T]
            wm = mm[u, 0:main].rearrange("(p c) -> p c", p=P)
            m0 = mref0[u, 0:main].rearrange("(p c) -> p c", p=P)
            m1 = mref1[u, 0:main].rearrange("(p c) -> p c", p=P)
            e_main8 = emb8[u, 0:main, :].rearrange("(p c) d -> p c d", p=P)
            e_main16 = emb16[u, 0:main, :].rearrange("(p c) d -> p c d", p=P)

            wo1p = spool.tile([P, 16], f32, tag="wo1p")   # per-tile sum wo1
            wp = spool.tile([P, 16], f32, tag="wp")       # per-tile sum w
            gp = psum.tile([P, 120], f32, tag="g")

            first = True
            tl = [(t * ew, ew, upat[t]) for t in range(NFULL_T)]
            if REM:
                tl.append((NFULL_T * ew, REM, "c"))
            gps_mm = []
            for ti, (co, cw, cls) in enumerate(tl):
                # -- per-tile prep: w / mref slices, mask, interleaved wo1w --
                w_sl = wpool.tile([P, cw], bf16, tag="w")
                nc.scalar.dma_start(out=w_sl[:], in_=wm[:, co : co + cw])
                mr0 = wpool.tile([P, cw], bf16, tag="mr0")
                nc.scalar.dma_start(out=mr0[:], in_=m0[:, co : co + cw])
                mr1 = wpool.tile([P, cw], bf16, tag="mr1")
                nc.scalar.dma_start(out=mr1[:], in_=m1[:, co : co + cw])
                mask = wpool.tile([P, cw], bf16, tag="mask")
                nc.vector.tensor_tensor(
                    mask[:], mr1[:], mr0[:], mybir.AluOpType.is_gt
                )
                wo1w = wpool.tile([P, cw * 2], bf16, tag="wo1w")
                w2 = wo1w[:].rearrange("p (c s) -> p c s", s=2)
                if ACC:
                    nc.vector.tensor_tensor_reduce(
                        w2[:, :, 0], w_sl[:], mask[:], 1.0, 0.0,
                        mybir.AluOpType.mult, mybir.AluOpType.add,
                        wo1p[:, ti : ti + 1],
                    )
                else:
                    nc.vector.tensor_mul(w2[:, :, 0], w_sl[:], mask[:])
                    nc.vector.tensor_reduce(
                        wo1p[:, ti : ti + 1], w2[:, :, 0].unsqueeze(1),
                        mybir.AxisListType.X, mybir.AluOpType.add,
                    )
                nc.vector.tensor_copy(w2[:, :, 1], w_sl[:])
                nc.vector.tensor_reduce(
                    wp[:, ti : ti + 1], w_sl[:].unsqueeze(1),
                    mybir.AxisListType.X, mybir.AluOpType.add,
                )

                # -- E tile --
                et = epool.tile([P, cw * D], bf16, tag="e")
                e3 = et[:].rearrange("p (c d) -> p c d", d=D)
                if cls == "p":
                    nc.sync.dma_start(out=e3[:], in_=e_main16[:, co : co + cw, :])
                else:
                    nc.gpsimd.dma_start(out=e3[:], in_=e_main8[:, co : co + cw, :])

                # -- L tile --
                lt = lpool.tile([P, cw * D1 + 2], bf16, tag="l")
                l3 = lt[:, 0 : cw * D1].rearrange("p (c e) -> p c e", e=D1)
                nc.vector.memset(lt[:, cw * D1 : cw * D1 + 2], 0.0)
                wr = wrpool.tile([P, cw * OMEGA], bf16, tag="wr")
                wr3 = wr[:].rearrange("p (c d) -> p c d", d=OMEGA)
                wsl8 = w_sl[:].unsqueeze(2).broadcast_to([P, cw, OMEGA])
                nc.scalar.activation(
                    wr3[:], wsl8, mybir.ActivationFunctionType.Copy
                )
                if cls == "G":
                    wsl = w_sl[:].unsqueeze(2).broadcast_to([P, cw, D])
                    nc.gpsimd.tensor_mul(l3[:, :, 0:D], e3[:], wsl)
                else:
                    for j in range(D // OMEGA):
                        nc.vector.tensor_mul(
                            l3[:, :, j * OMEGA : (j + 1) * OMEGA],
                            e3[:, :, j * OMEGA : (j + 1) * OMEGA],
                            wr3[:],
                        )
                nc.vector.tensor_copy(l3[:, :, D : D + 2], w2[:, :, :])

                # -- FWL-window matmuls --
                nb = cw // 3
                mms = []
                for bb in range(nb):
                    mms.append((
                        lt[:, bb * 3 * D1 : bb * 3 * D1 + 128],
                        et[:, bb * 3 * D : (bb + 1) * 3 * D],
                        False,
                    ))
                for c in range(nb * 3, cw):
                    mms.append((
                        lt[:, c * D1 : (c + 1) * D1],
                        et[:, c * D : (c + 1) * D],
                        True,
                    ))
                if cls == "G":
                    gps_mm.extend(mms)
                    continue
                for lhsT, rhs, single in mms:
                    if single:
                        nc.tensor.matmul(
                            gp[0:D1, 0:D], lhsT, rhs,
                            start=False, stop=False,
                            tile_position=(0, 0), skip_group_check=True,
                        )
                    else:
                        nc.tensor.matmul(
                            gp[:, :], lhsT, rhs,
                            start=first, stop=False, skip_group_check=True,
                        )
                        first = False

            # deferred G-tile matmuls (L built by GpSimd long before)
            for lhsT, rhs, single in gps_mm:
                if single:
                    nc.tensor.matmul(
                        gp[0:D1, 0:D], lhsT, rhs,
                        start=False, stop=False,
                        tile_position=(0, 0), skip_group_check=True,
                    )
                else:
                    nc.tensor.matmul(
                        gp[:, :], lhsT, rhs,
                        start=False, stop=False, skip_group_check=True,
                    )

            # ---- tail chunk ----
            wtl = spool.tile([P, 1], bf16, tag="wtl")
            nc.scalar.dma_start(out=wtl[0:tail, :], in_=mm[u, main:ft].unsqueeze(1))
            mrtl0 = spool.tile([P, 1], bf16, tag="mrtl0")
            nc.scalar.dma_start(out=mrtl0[0:tail, :], in_=mref0[u, main:ft].unsqueeze(1))
            mrtl1 = spool.tile([P, 1], bf16, tag="mrtl1")
            nc.scalar.dma_start(out=mrtl1[0:tail, :], in_=mref1[u, main:ft].unsqueeze(1))
            masktl = spool.tile([P, 1], bf16, tag="masktl")
            nc.vector.tensor_tensor(
                masktl[0:tail, :], mrtl1[0:tail, :], mrtl0[0:tail, :],
                mybir.AluOpType.is_gt,
            )
            wo1tl = spool.tile([P, S], bf16, tag="wo1tl")
            nc.vector.tensor_mul(wo1tl[0:tail, 0:1], wtl[0:tail, :], masktl[0:tail, :])
            nc.vector.tensor_copy(wo1tl[0:tail, 1:2], wtl[0:tail, :])
            etl = spool.tile([P, D], bf16, tag="etl")
            nc.gpsimd.dma_start(out=etl[0:tail, :], in_=emb8[u, main:ft, :])
            ltl = spool.tile([P, D1], bf16, tag="ltl")
            nc.vector.tensor_mul(
                ltl[0:tail, 0:D],
                etl[0:tail, :],
                wtl[0:tail, :].broadcast_to([tail, D]),
            )
            nc.vector.tensor_copy(ltl[0:tail, D : D + S], wo1tl[0:tail, :])
            nc.tensor.matmul(
                gp[0:D1, 0:D], ltl[0:tail, :], etl[0:tail, :],
                start=False, stop=True,
                tile_position=(0, 0), skip_group_check=True,
            )

            # ---- assemble b sums ----
            wored = spool.tile([P, S], f32, tag="wored")
            nc.vector.tensor_reduce(
                wored[:, 0:1], wo1p[:, 0:NTT].unsqueeze(1),
                mybir.AxisListType.X, mybir.AluOpType.add,
            )
            nc.vector.tensor_reduce(
                wored[:, 1:2], wp[:, 0:NTT].unsqueeze(1),
                mybir.AxisListType.X, mybir.AluOpType.add,
            )
            nc.vector.tensor_add(wored[0:tail, :], wored[0:tail, :], wo1tl[0:tail, :])
            nc.scalar.dma_start(out=b_out[u, :, :], in_=wored[:])
            gsb = spool.tile([P, 120], f32, tag="gsb")
            nc.scalar.activation(gsb[:], gp[:], mybir.ActivationFunctionType.Copy)
            nc.scalar.dma_start(out=g_out[u, :, :], in_=gsb[:])

    nc.compile()
    return nc


def _finish_host_v2(g_all, b_all):
    """g_all: [N, 128, 120] block-diagonal dumps, b_all: [N, P, 2] -> loss."""
    g = g_all.astype(np.float64)
    G = (
        g[:, 0:D1V, 0:D]
        + g[:, D1V : 2 * D1V, D : 2 * D]
        + g[:, 2 * D1V : 3 * D1V, 2 * D : 3 * D]
    )  # [N, 42, 40]
    b = b_all.astype(np.float64).sum(axis=1)  # [N, 2] = (b1, M)
    A = G[:, 0:D, :]
    C1 = G[:, D, :]
    t = G[:, D + 1, :]
    C0 = t - C1
    b1 = b[:, 0]
    M = b[:, 1]
    b0 = M - b1
    a2 = (A**2).sum(axis=(1, 2))
    c2 = (C0**2).sum(axis=1) + (C1**2).sum(axis=1)
    loss = (a2 + b0**2 + b1**2 - 2.0 * c2) / (M * M * T)
    return np.asarray(loss.mean(), dtype=np.float32)


D1V = D + S


def _get_program(key):
    if key not in _prog_cache:
        if key[-1] == "v2":
            _prog_cache[key] = _build_v2(*key[:-1], pat=PAT2)
        elif key[-1] == "perm":
            _prog_cache[key] = _build_perm(*key[:-1])
        elif key[-1] == "tile2":
            _prog_cache[key] = _build_tile2(
                *key[:-1], ng_pool=NG_POOL, prep_pool=PREP_POOL
            )
        else:
            _prog_cache[key] = _build_program(*key)
    return _prog_cache[key]


def _finish_host(g_all, b_all):
    """g_all: [N, 42, 40] (or [N, 2, 42, 40]), b_all: [N, P, 2] -> loss."""
    if g_all.ndim == 4:
        g_all = g_all.sum(axis=1, dtype=np.float64)
    g = g_all.astype(np.float64)
    b = b_all.astype(np.float64).sum(axis=1)  # [N, 2]
    a2 = (g[:, 0:D, :] ** 2).sum(axis=(1, 2))
    c2 = (g[:, D : D + S, :] ** 2).sum(axis=(1, 2))
    b2 = (b**2).sum(axis=1)
    m = b.sum(axis=1)
    loss = (a2 + b2 - 2.0 * c2) / (m * m * T)
    return np.asarray(loss.mean(), dtype=np.float32)


def _install_trace_shim():
    """Provide the antenv.axon_hooks module bass_utils expects for NTFF
    profiling under axon (this image's antenv lacks it)."""
    import sys as _sys
    import types

    if "antenv.axon_hooks" in _sys.modules:
        return
    try:
        from trn_agent_boot.trn_boot import _ntff_profile_via_ctypes

        hook = _ntff_profile_via_ctypes("/opt/axon/libaxon_pjrt.so")
    except Exception:
        hook = None
    mod = types.ModuleType("antenv.axon_hooks")
    mod.get_axon_ntff_profile_hook = lambda: hook
    mod.set_axon_ntff_profile_hook = lambda h: None
    _sys.modules["antenv.axon_hooks"] = mod


def kernel(embedding, magnitude_ref, magnitude_mix):
    from concourse.bass_utils import run_bass_kernel_spmd

    global LAST_EXEC_NS
    mref = np.ascontiguousarray(magnitude_ref, dtype=np.float32).reshape(N_FULL, FT, S)
    mm = np.ascontiguousarray(magnitude_mix, dtype=np.float32).reshape(N_FULL, FT)
    core_ids = list(range(NCORES))

    if MODE == "v2":
        import ml_dtypes

        emb32 = np.ascontiguousarray(embedding, dtype=np.float32)
        emb8 = emb32.astype(ml_dtypes.float8_e4m3fn)
        emb16 = emb32.astype(ml_dtypes.bfloat16)
        mm16 = mm.astype(ml_dtypes.bfloat16)
        mref16 = mref.astype(ml_dtypes.bfloat16)
        mref0 = np.ascontiguousarray(mref16[:, :, 0])
        mref1 = np.ascontiguousarray(mref16[:, :, 1])
        nc = _get_program((NPER, CPP, EW2, TAIL, "v2"))
        in_maps = [
            {
                "emb8": emb8[i * NPER : (i + 1) * NPER],
                "emb16": emb16[i * NPER : (i + 1) * NPER],
                "mm": mm16[i * NPER : (i + 1) * NPER],
                "mref0": mref0[i * NPER : (i + 1) * NPER],
                "mref1": mref1[i * NPER : (i + 1) * NPER],
            }
            for i in core_ids
        ]
    elif MODE == "perm":
        import ml_dtypes

        emb32 = np.ascontiguousarray(embedding, dtype=np.float32)
        emb_p = (
            emb32[:, :MAIN, :]
            .reshape(N_FULL, P, CPP, D)
            .transpose(0, 1, 3, 2)
            .astype(ml_dtypes.bfloat16)
        )
        emb_t = emb32[:, MAIN:, :].astype(ml_dtypes.bfloat16)
        nc = _get_program((NPER, CPP, EW, CB, TAIL, "perm"))
        in_maps = [
            {
                "emb_p": emb_p[i * NPER : (i + 1) * NPER],
                "emb_t": emb_t[i * NPER : (i + 1) * NPER],
                "mm": mm[i * NPER : (i + 1) * NPER],
                "mref": mref[i * NPER : (i + 1) * NPER],
            }
            for i in core_ids
        ]
    elif MODE == "tile2":
        import ml_dtypes

        emb = np.ascontiguousarray(embedding).astype(ml_dtypes.bfloat16)
        mref = mref.astype(np.float16)
        mm = mm.astype(np.float16)
        nc = _get_program((NPER, CPP, EW, CB, TAIL, "tile2"))
        in_maps = [
            {
                "emb": emb[i * NPER : (i + 1) * NPER],
                "mm": mm[i * NPER : (i + 1) * NPER],
                "mref": mref[i * NPER : (i + 1) * NPER],
            }
            for i in core_ids
        ]
    else:
        if MODE == "bf16host":
            import ml_dtypes

            emb = np.ascontiguousarray(embedding).astype(ml_dtypes.bfloat16)
        else:
            emb = np.ascontiguousarray(embedding, dtype=np.float32)
        nc = _get_program((NPER, CPP, CB, NGROUPS, TAIL, MODE))
        in_maps = [
            {
                "emb": emb[i * NPER : (i + 1) * NPER],
                "mm": mm[i * NPER : (i + 1) * NPER],
                "mref": mref[i * NPER : (i + 1) * NPER],
            }
            for i in core_ids
        ]
    trace = os.environ.get("DPCL_TRACE", "0") == "1"
    if trace:
        _install_trace_shim()
    res = None
    for attempt in range(3):
        try:
            res = run_bass_kernel_spmd(nc, in_maps, core_ids, trace=trace)
            break
        except Exception:
            if attempt == 2:
                raise
    assert res is not None
    LAST_EXEC_NS = res.exec_time_ns

    g_all = np.concatenate([r["g_out"] for r in res.results], axis=0)
    b_all = np.concatenate([r["b_out"] for r in res.results], axis=0)
    if MODE == "v2":
        return _finish_host_v2(g_all, b_all)
    return _finish_host(g_all, b_all)



# revision 13
# speedup vs baseline: 1.2876x; 1.2876x over previous
"""DPCL objective (deep-clustering loss) on 8 Trainium2 NeuronCores.

Sharding: pure data parallel -- batch dim N=16 -> 2 utterances per core.
For each utterance the loss reduces to the 42x40 weighted Gram matrix

    G = [W*E | wo0 | wo1]^T @ E        (contraction over FT = 154200)

where E is the (FT, 40) embedding, W = diag(magnitude_mix row), and
wo_s = magnitude_mix * onehot_s(argmax(magnitude_ref)).  Because the
weights enter the affinity bilinearly, scaling ONE matmul operand by the
un-normalized magnitudes is enough (no sqrt, no normalization on device):

    A  = out^T out = G[0:40] / M,   C^T = G[40:42] / M,
    B  = diag(b0, b1)/M,  b_s = sum_k wo_s[k],  M = b0 + b1
    loss_n = (||A||^2 + ||B||^2 - 2||C||^2) / (M^2 T)

Host work is limited to slicing inputs per core, casting the embedding to
bf16 (PSUM accumulation stays fp32; end-to-end rel err ~6e-5), and
combining the 16 tiny Gram matrices into the scalar loss.

Device kernel (default "tile2" build), per core:
  - FT layout 154200 = 128*1204 + 88: partition p owns rows
    [p*1204, (p+1)*1204), so every DMA is per-partition contiguous with
    zero host-side copies; the 88-row tail is one extra small-K matmul.
  - PE: 2410 accumulating 128x42x40 matmuls, alternating between two
    column-tile positions (0,0)/(0,64) of the 128x128 array so pairs of
    chunks run concurrently (two 42-col stationaries side by side); the
    two partial Grams are summed on the host.
  - DVE: the weighted copy W*E.  The d-broadcast weights are first
    materialized in (c,d) bf16 layout by the scalar engine ("wrep"),
    which makes the multiply an all-bf16 stride-1 op -> DVE packed 2x
    mode (1.95us vs 3.73us per 86-chunk group).
  - ACT: wrep builds, wo-column copies into the stationary, PSUM->SBUF.
  - DMA: E tiles spread over three independent queue rows (SWDGE q0 via
    GpSimd fire-and-forget, SP HWDGE q1, ACT HWDGE q10) -- a single HWDGE
    ring blocks its issuing engine for the whole transfer.
Measured ~145-165 us HW time (8 cores, ~28.7 MB HBM reads/core).
"""

import os
import sys
import numpy as np
from contextlib import ExitStack

sys.path.insert(0, "/opt/trn_rl_repo")

N_FULL = 16
F, T, S, D = 257, 600, 2, 40
FT = F * T                      # 154200
NCORES = 8
NPER = N_FULL // NCORES         # 2 utterances per core
P = 128

# full-size FT decomposition: FT = P*CPP + TAIL
CPP = FT // P                   # 1204 columns per partition (main part)
MAIN = P * CPP                  # 154112
TAIL = FT - MAIN                # 88
CB = 86                         # chunks per group
NGROUPS = CPP // CB             # 14

# matmul operand dtype / transfer strategy:
#   "f32"      - fp32 matmuls (4 cyc/row), fp32 DMA
#   "bf16"     - bf16 matmuls, cast-during-DMA (SWDGE), fp32 HBM reads
#   "bf16host" - bf16 matmuls, embedding pre-cast on host (halves HBM reads)
#   "perm"     - bf16host + host-permuted [P, D, c] layout (packed 2x DVE
#                weighted-copy) + PE column-tiling (2 concurrent chunks)
MODE = os.environ.get("DPCL_MODE", "v2")
EW = int(os.environ.get("DPCL_EW", "172"))  # E-tile chunk width
EBUFS = int(os.environ.get("DPCL_EBUFS", "5"))
PBUFS = int(os.environ.get("DPCL_PBUFS", "2"))
NG_POOL = int(os.environ.get("DPCL_NGPOOL", "0"))      # WE groups on GpSimd
PREP_POOL = os.environ.get("DPCL_PREPPOOL", "0") == "1"  # mask prep on GpSimd
ERINGS = int(os.environ.get("DPCL_ERINGS", "3"))
BDVE = int(os.environ.get("DPCL_BDVE", "2"))  # of each 7 wrep builds, this many on DVE
K7 = int(os.environ.get("DPCL_K7", "7"))  # of each 7 tiles, this many use ACT-wrep
# rank of each position in the 7-cycle: positions with rank < K7 use wrep.
WREP_PAT = (
    [0, 1, 5, 2, 3, 6, 4]
    if os.environ.get("DPCL_PAT", "id") == "il"
    else [0, 1, 2, 3, 4, 5, 6]
)

LAST_EXEC_NS = None

_prog_cache = {}


def _build_program(nper, cpp, cb, ngroups, tail, mode):
    import concourse.bass as bass
    import concourse.bacc as bacc
    import concourse.tile as tile
    from concourse import mybir

    f32 = mybir.dt.float32
    dmm = f32 if mode == "f32" else mybir.dt.bfloat16
    ft = P * cpp + tail
    main = P * cpp
    assert ngroups * cb == cpp

    nc = bacc.Bacc(
        "TRN2", target_bir_lowering=False, debug=False, num_devices=NCORES
    )
    emb_dt = dmm if mode == "bf16host" else f32
    emb = nc.declare_dram_parameter("emb", [nper, ft, D], emb_dt, isOutput=False)
    mm = nc.declare_dram_parameter("mm", [nper, ft], f32, isOutput=False)
    mref = nc.declare_dram_parameter("mref", [nper, ft, S], f32, isOutput=False)
    g_out = nc.declare_dram_parameter("g_out", [nper, D + S, D], f32, isOutput=True)
    b_out = nc.declare_dram_parameter("b_out", [nper, P, S], f32, isOutput=True)

    # engine used for the big E loads (SWDGE supports dtype-cast during DMA)
    if mode == "bf16":
        e_dma = lambda out, in_: nc.gpsimd.dma_start(out=out, in_=in_)
    else:
        e_dma = lambda out, in_: nc.sync.dma_start(out=out, in_=in_)
    # in bf16 (cast-DMA) mode GpSimd is busy generating descriptors; otherwise
    # split the big weighted-copy work between DVE and GpSimd
    split_we = mode != "bf16"

    with tile.TileContext(nc) as tc, ExitStack() as ctx:
        wpool = ctx.enter_context(tc.tile_pool(name="wpool", bufs=2))
        epool = ctx.enter_context(tc.tile_pool(name="epool", bufs=3))
        lpool = ctx.enter_context(tc.tile_pool(name="lpool", bufs=3))
        spool = ctx.enter_context(tc.tile_pool(name="spool", bufs=2))
        psum = ctx.enter_context(tc.tile_pool(name="psum", bufs=2, space="PSUM"))

        for u in range(nper):
            # ---- per-row weight / mask prep (all [128, cpp]) ----
            w_t = wpool.tile([P, cpp], f32, tag="w")
            nc.sync.dma_start(
                out=w_t[:], in_=mm[u, 0:main].rearrange("(p c) -> p c", p=P)
            )
            mr_t = wpool.tile([P, cpp * S], f32, tag="mr")
            nc.sync.dma_start(
                out=mr_t[:],
                in_=mref[u, 0:main, :].rearrange("(p c) s -> p (c s)", p=P),
            )
            mr3 = mr_t[:].rearrange("p (c s) -> p c s", s=S)
            mask_t = wpool.tile([P, cpp], f32, tag="mask")
            # mask = 1.0 where speaker-1 magnitude wins the argmax
            nc.vector.tensor_tensor(
                mask_t[:], mr3[:, :, 1], mr3[:, :, 0], mybir.AluOpType.is_gt
            )
            wo_t = wpool.tile([P, S * cpp], f32, tag="wo")  # [wo0 | wo1]
            nc.vector.tensor_mul(wo_t[:, cpp : 2 * cpp], w_t[:], mask_t[:])
            nc.vector.tensor_sub(wo_t[:, 0:cpp], w_t[:], wo_t[:, cpp : 2 * cpp])
            wo3 = wo_t[:].rearrange("p (s c) -> p c s", s=S)

            wored = spool.tile([P, S], f32, tag="wored")
            nc.vector.tensor_reduce(
                wored[:],
                wo_t[:].rearrange("p (s c) -> p s c", s=S),
                mybir.AxisListType.X,
                mybir.AluOpType.add,
            )

            # ---- tail prep ([tail, *]) ----
            wtl = spool.tile([P, 1], f32, tag="wtl")
            nc.sync.dma_start(out=wtl[0:tail, :], in_=mm[u, main:ft].unsqueeze(1))
            mrtl = spool.tile([P, S], f32, tag="mrtl")
            nc.sync.dma_start(out=mrtl[0:tail, :], in_=mref[u, main:ft, :])
            masktl = spool.tile([P, 1], f32, tag="masktl")
            nc.vector.tensor_tensor(
                masktl[0:tail, :],
                mrtl[0:tail, 1:2],
                mrtl[0:tail, 0:1],
                mybir.AluOpType.is_gt,
            )
            wotl = spool.tile([P, S], f32, tag="wotl")
            nc.vector.tensor_mul(wotl[0:tail, 1:2], wtl[0:tail, :], masktl[0:tail, :])
            nc.vector.tensor_sub(wotl[0:tail, 0:1], wtl[0:tail, :], wotl[0:tail, 1:2])
            nc.vector.tensor_add(wored[0:tail, :], wored[0:tail, :], wotl[0:tail, :])
            nc.sync.dma_start(out=b_out[u, :, :], in_=wored[:])

            # ---- Gram accumulation ----
            gp = psum.tile([D + S, D], f32, tag="g")
            e_main = emb[u, 0:main, :].rearrange("(p c) d -> p c d", p=P)
            for g in range(ngroups):
                et = epool.tile([P, cb * D], dmm, tag="e")
                e3 = et[:].rearrange("p (c d) -> p c d", d=D)
                e_dma(e3[:], e_main[:, g * cb : (g + 1) * cb, :])

                lt = lpool.tile([P, cb * (D + S)], dmm, tag="l")
                l3 = lt[:].rearrange("p (c e) -> p c e", e=D + S)
                # weighted copy of E into the stationary operand
                wslice = w_t[:, g * cb : (g + 1) * cb].unsqueeze(2).broadcast_to(
                    [P, cb, D]
                )
                weng = nc.gpsimd if (split_we and g % 2 == 1) else nc.vector
                weng.tensor_mul(l3[:, :, 0:D], e3[:], wslice)
                # masked-weight columns (wo0, wo1)
                weng.tensor_copy(
                    l3[:, :, D : D + S], wo3[:, g * cb : (g + 1) * cb, :]
                )
                for c in range(cb):
                    nc.tensor.matmul(
                        gp[:],
                        lt[:, c * (D + S) : (c + 1) * (D + S)],
                        et[:, c * D : (c + 1) * D],
                        start=(g == 0 and c == 0),
                        stop=False,
                    )

            # tail chunk (contraction dim = tail)
            etl = spool.tile([P, D], dmm, tag="etl")
            e_dma(etl[0:tail, :], emb[u, main:ft, :])
            ltl = spool.tile([P, D + S], dmm, tag="ltl")
            nc.vector.tensor_mul(
                ltl[0:tail, 0:D],
                etl[0:tail, :],
                wtl[0:tail, :].broadcast_to([tail, D]),
            )
            nc.vector.tensor_copy(ltl[0:tail, D : D + S], wotl[0:tail, :])
            nc.tensor.matmul(
                gp[:], ltl[0:tail, :], etl[0:tail, :], start=False, stop=True
            )

            gsb = spool.tile([D + S, D], f32, tag="gsb")
            nc.scalar.activation(gsb[:], gp[:], mybir.ActivationFunctionType.Copy)
            nc.sync.dma_start(out=g_out[u, :, :], in_=gsb[:])

    nc.compile()
    return nc


def _build_perm(nper, cpp, ew, cb, tail):
    """Permuted-layout bf16 build: E arrives as [nper, P, D, cpp] so the
    weighted copy hits DVE's packed 2x mode, and chunks alternate between
    two PE column-tile positions (the 42-col stationary only uses a third
    of the array)."""
    import concourse.bacc as bacc
    import concourse.tile as tile
    from concourse import mybir

    f32 = mybir.dt.float32
    bf16 = mybir.dt.bfloat16
    ft = P * cpp + tail
    main = P * cpp
    ntiles = cpp // ew
    gpe = ew // cb
    assert ntiles * ew == cpp and gpe * cb == ew and cb % 2 == 0

    nc = bacc.Bacc(
        "TRN2", target_bir_lowering=False, debug=False, num_devices=NCORES
    )
    emb_p = nc.declare_dram_parameter("emb_p", [nper, P, D, cpp], bf16, isOutput=False)
    emb_t = nc.declare_dram_parameter("emb_t", [nper, tail, D], bf16, isOutput=False)
    mm = nc.declare_dram_parameter("mm", [nper, ft], f32, isOutput=False)
    mref = nc.declare_dram_parameter("mref", [nper, ft, S], f32, isOutput=False)
    g_out = nc.declare_dram_parameter(
        "g_out", [nper, 2, D + S, D], f32, isOutput=True
    )
    b_out = nc.declare_dram_parameter("b_out", [nper, P, S], f32, isOutput=True)

    with tile.TileContext(nc) as tc, ExitStack() as ctx:
        wpool = ctx.enter_context(tc.tile_pool(name="wpool", bufs=2))
        epool = ctx.enter_context(tc.tile_pool(name="epool", bufs=3))
        lpool = ctx.enter_context(tc.tile_pool(name="lpool", bufs=3))
        spool = ctx.enter_context(tc.tile_pool(name="spool", bufs=2))
        psum = ctx.enter_context(tc.tile_pool(name="psum", bufs=2, space="PSUM"))

        for u in range(nper):
            # ---- per-row weight / mask prep (all [128, cpp], fp32) ----
            w_t = wpool.tile([P, cpp], f32, tag="w")
            nc.sync.dma_start(
                out=w_t[:], in_=mm[u, 0:main].rearrange("(p c) -> p c", p=P)
            )
            mr_t = wpool.tile([P, cpp * S], f32, tag="mr")
            nc.sync.dma_start(
                out=mr_t[:],
                in_=mref[u, 0:main, :].rearrange("(p c) s -> p (c s)", p=P),
            )
            mr3 = mr_t[:].rearrange("p (c s) -> p c s", s=S)
            mask_t = wpool.tile([P, cpp], f32, tag="mask")
            nc.vector.tensor_tensor(
                mask_t[:], mr3[:, :, 1], mr3[:, :, 0], mybir.AluOpType.is_gt
            )
            wo_t = wpool.tile([P, S * cpp], f32, tag="wo")  # [wo0 | wo1]
            nc.vector.tensor_mul(wo_t[:, cpp : 2 * cpp], w_t[:], mask_t[:])
            nc.vector.tensor_sub(wo_t[:, 0:cpp], w_t[:], wo_t[:, cpp : 2 * cpp])
            wo_sc = wo_t[:].rearrange("p (s c) -> p s c", s=S)
            w_bf = wpool.tile([P, cpp], bf16, tag="wbf")
            nc.vector.tensor_copy(w_bf[:], w_t[:])

            wored = spool.tile([P, S], f32, tag="wored")
            nc.vector.tensor_reduce(
                wored[:],
                wo_t[:].rearrange("p (s c) -> p s c", s=S),
                mybir.AxisListType.X,
                mybir.AluOpType.add,
            )

            # ---- tail prep ----
            wtl = spool.tile([P, 1], f32, tag="wtl")
            nc.sync.dma_start(out=wtl[0:tail, :], in_=mm[u, main:ft].unsqueeze(1))
            mrtl = spool.tile([P, S], f32, tag="mrtl")
            nc.sync.dma_start(out=mrtl[0:tail, :], in_=mref[u, main:ft, :])
            masktl = spool.tile([P, 1], f32, tag="masktl")
            nc.vector.tensor_tensor(
                masktl[0:tail, :],
                mrtl[0:tail, 1:2],
                mrtl[0:tail, 0:1],
                mybir.AluOpType.is_gt,
            )
            wotl = spool.tile([P, S], f32, tag="wotl")
            nc.vector.tensor_mul(wotl[0:tail, 1:2], wtl[0:tail, :], masktl[0:tail, :])
            nc.vector.tensor_sub(wotl[0:tail, 0:1], wtl[0:tail, :], wotl[0:tail, 1:2])
            nc.vector.tensor_add(wored[0:tail, :], wored[0:tail, :], wotl[0:tail, :])
            nc.sync.dma_start(out=b_out[u, :, :], in_=wored[:])

            # ---- Gram accumulation, two column-tile positions ----
            gp = psum.tile([P, D], f32, tag="g")
            started = [False, False]
            for t in range(ntiles):
                et = epool.tile([P, D * ew], bf16, tag="e")
                e3 = et[:].rearrange("p (d c) -> p d c", c=ew)
                nc.sync.dma_start(
                    out=e3[:], in_=emb_p[u, :, :, t * ew : (t + 1) * ew]
                )
                for gc in range(gpe):
                    co = gc * cb
                    lt = lpool.tile([P, cb * (D + S)], bf16, tag="l")
                    l3 = lt[:].rearrange("p (e c) -> p e c", c=cb)
                    wsl = (
                        w_bf[:, t * ew + co : t * ew + co + cb]
                        .unsqueeze(1)
                        .broadcast_to([P, D, cb])
                    )
                    nc.vector.tensor_mul(l3[:, 0:D, :], e3[:, :, co : co + cb], wsl)
                    nc.vector.tensor_copy(
                        l3[:, D : D + S, :],
                        wo_sc[:, :, t * ew + co : t * ew + co + cb],
                    )
                    for c in range(cb):
                        k = t * ew + co + c
                        par = k % 2
                        pb = 64 * par
                        st = not started[par]
                        started[par] = True
                        nc.tensor.matmul(
                            gp[pb : pb + D + S, :],
                            l3[:, :, c : c + 1],
                            e3[:, :, co + c : co + c + 1],
                            start=st,
                            stop=(par == 1 and k == cpp - 1),
                            tile_position=(0, pb),
                            skip_group_check=True,
                        )

            # tail chunk -> position 0 accumulator, closes its group
            etl = spool.tile([P, D], bf16, tag="etl")
            nc.sync.dma_start(out=etl[0:tail, :], in_=emb_t[u, :, :])
            ltl = spool.tile([P, D + S], bf16, tag="ltl")
            nc.vector.tensor_mul(
                ltl[0:tail, 0:D],
                etl[0:tail, :],
                wtl[0:tail, :].broadcast_to([tail, D]),
            )
            nc.vector.tensor_copy(ltl[0:tail, D : D + S], wotl[0:tail, :])
            nc.tensor.matmul(
                gp[0 : D + S, :],
                ltl[0:tail, :],
                etl[0:tail, :],
                start=False,
                stop=True,
                tile_position=(0, 0),
                skip_group_check=True,
            )

            gsb = spool.tile([P, D], f32, tag="gsb")
            nc.scalar.activation(
                gsb[0 : D + S, :], gp[0 : D + S, :], mybir.ActivationFunctionType.Copy
            )
            nc.scalar.activation(
                gsb[64 : 64 + D + S, :],
                gp[64 : 64 + D + S, :],
                mybir.ActivationFunctionType.Copy,
            )
            nc.sync.dma_start(out=g_out[u, 0, :, :], in_=gsb[0 : D + S, :])
            nc.sync.dma_start(out=g_out[u, 1, :, :], in_=gsb[64 : 64 + D + S, :])

    nc.compile()
    return nc


def _build_tile2(nper, cpp, ew, cb, tail, ng_pool=0, prep_pool=True):
    """Contiguous (c,d) layouts for all PE operands + 2-way PE column
    tiling + DVE/GpSimd split of the weighted copy + dual HWDGE rings."""
    import concourse.bacc as bacc
    import concourse.tile as tile
    from concourse import mybir

    f32 = mybir.dt.float32
    bf16 = mybir.dt.bfloat16
    ft = P * cpp + tail
    main = P * cpp
    ntiles = cpp // ew
    gpe = ew // cb
    assert ntiles * ew == cpp and gpe * cb == cb * gpe and gpe * cb == ew

    nc = bacc.Bacc(
        "TRN2", target_bir_lowering=False, debug=False, num_devices=NCORES
    )
    emb = nc.declare_dram_parameter("emb", [nper, ft, D], bf16, isOutput=False)
    f16 = mybir.dt.float16
    mm = nc.declare_dram_parameter("mm", [nper, ft], f16, isOutput=False)
    mref = nc.declare_dram_parameter("mref", [nper, ft, S], f16, isOutput=False)
    g_out = nc.declare_dram_parameter(
        "g_out", [nper, 2, D + S, D], f32, isOutput=True
    )
    b_out = nc.declare_dram_parameter("b_out", [nper, P, S], f32, isOutput=True)

    total_groups = nper * ntiles * gpe

    with tile.TileContext(nc) as tc, ExitStack() as ctx:
        wpool = ctx.enter_context(tc.tile_pool(name="wpool", bufs=2))
        ppool = ctx.enter_context(tc.tile_pool(name="ppool", bufs=PBUFS))
        epool = ctx.enter_context(tc.tile_pool(name="epool", bufs=EBUFS))
        lpool = ctx.enter_context(tc.tile_pool(name="lpool", bufs=3))
        wrpool = ctx.enter_context(tc.tile_pool(name="wrpool", bufs=2))
        spool = ctx.enter_context(tc.tile_pool(name="spool", bufs=2))
        psum = ctx.enter_context(tc.tile_pool(name="psum", bufs=2, space="PSUM"))

        gi = 0  # global group index for the DVE/GpSimd split
        prep = {}
        for u in range(nper):
            # ---- per-row weight / mask prep (fp32 [128, cpp]) ----
            w_t = wpool.tile([P, cpp], f16, tag="w")
            nc.sync.dma_start(
                out=w_t[:], in_=mm[u, 0:main].rearrange("(p c) -> p c", p=P)
            )
            mr_t = ppool.tile([P, cpp * S], f16, tag="mr")
            nc.sync.dma_start(
                out=mr_t[:],
                in_=mref[u, 0:main, :].rearrange("(p c) s -> p (c s)", p=P),
            )
            mr3 = mr_t[:].rearrange("p (c s) -> p c s", s=S)
            peng = nc.gpsimd if prep_pool else nc.vector
            mask_t = ppool.tile([P, cpp], f32, tag="mask")
            nc.vector.tensor_tensor(
                mask_t[:], mr3[:, :, 1], mr3[:, :, 0], mybir.AluOpType.is_gt
            )
            wo_t = wpool.tile([P, S * cpp], f32, tag="wo")  # [wo0 | wo1]
            peng.tensor_mul(wo_t[:, cpp : 2 * cpp], w_t[:], mask_t[:])
            peng.tensor_sub(wo_t[:, 0:cpp], w_t[:], wo_t[:, cpp : 2 * cpp])
            wo_sc = wo_t[:].rearrange("p (s c) -> p s c", s=S)

            wored = spool.tile([P, S], f32, tag="wored")
            nc.vector.tensor_reduce(
                wored[:],
                wo_t[:].rearrange("p (s c) -> p s c", s=S),
                mybir.AxisListType.X,
                mybir.AluOpType.add,
            )

            # ---- tail prep ----
            wtl = spool.tile([P, 1], f16, tag="wtl")
            nc.sync.dma_start(out=wtl[0:tail, :], in_=mm[u, main:ft].unsqueeze(1))
            mrtl = spool.tile([P, S], f16, tag="mrtl")
            nc.sync.dma_start(out=mrtl[0:tail, :], in_=mref[u, main:ft, :])
            masktl = spool.tile([P, 1], f32, tag="masktl")
            nc.vector.tensor_tensor(
                masktl[0:tail, :],
                mrtl[0:tail, 1:2],
                mrtl[0:tail, 0:1],
                mybir.AluOpType.is_gt,
            )
            wotl = spool.tile([P, S], f32, tag="wotl")
            nc.vector.tensor_mul(wotl[0:tail, 1:2], wtl[0:tail, :], masktl[0:tail, :])
            nc.vector.tensor_sub(wotl[0:tail, 0:1], wtl[0:tail, :], wotl[0:tail, 1:2])
            nc.vector.tensor_add(wored[0:tail, :], wored[0:tail, :], wotl[0:tail, :])
            nc.sync.dma_start(out=b_out[u, :, :], in_=wored[:])
            prep[u] = (w_t, wo_sc, wtl, wotl)

        for u in range(nper):
            w_t, wo_sc, wtl, wotl = prep[u]
            # ---- Gram accumulation ----
            gp = psum.tile([P, D], f32, tag="g")
            started = [False, False]
            e_main = emb[u, 0:main, :].rearrange("(p c) d -> p c d", p=P)
            for t in range(ntiles):
                et = epool.tile([P, ew * D], bf16, tag="e")
                e3 = et[:].rearrange("p (c d) -> p c d", d=D)
                # spread the big loads over three independent DMA queue rows:
                # SWDGE (q0, fire-and-forget after ~1us Q7 emission), the SP
                # HWDGE ring (q1) and the ACT HWDGE ring (q10)
                if ERINGS == 2:
                    ering = (nc.gpsimd, nc.sync)[t % 2]
                else:
                    ering = (nc.sync, nc.gpsimd, nc.scalar)[t % 3]
                ering.dma_start(out=e3[:], in_=e_main[:, t * ew : (t + 1) * ew, :])

                lt = lpool.tile([P, ew * (D + S)], bf16, tag="l")
                l3 = lt[:].rearrange("p (c e) -> p c e", e=D + S)
                wsl = (
                    w_t[:, t * ew : (t + 1) * ew]
                    .unsqueeze(2)
                    .broadcast_to([P, ew, D])
                )
                # For most tiles, materialize the d-broadcast weights in (c,d)
                # bf16 layout on the otherwise-idle ACT engine; the weighted
                # copy then runs all-bf16 step-1 => DVE packed 2x mode
                # (1.95us vs 3.73us per group).  The rest run the direct 1x
                # broadcast multiply on DVE, balancing ACT vs DVE.
                use_wrep = WREP_PAT[gi % 7] < K7
                if use_wrep:
                    wrt = wrpool.tile([P, ew * D], bf16, tag="wr")
                    wr3 = wrt[:].rearrange("p (c d) -> p c d", d=D)
                    if WREP_PAT[gi % 7] >= 7 - BDVE:
                        nc.vector.tensor_copy(wr3[:], wsl)
                    else:
                        nc.scalar.activation(
                            wr3[:], wsl, mybir.ActivationFunctionType.Copy
                        )
                # one wo-columns copy per tile (ACT, overhead-dominated)
                nc.vector.tensor_copy(
                    l3[:, :, D : D + S],
                    wo_sc[:, :, t * ew : (t + 1) * ew].transpose([0, 2, 1]),
                )
                nc.vector.tensor_mul(
                    l3[:, :, 0:D], e3[:], wr3[:] if use_wrep else wsl
                )
                for gc in range(gpe):
                    co = gc * cb
                    gi += 1
                    for c in range(cb):
                        k = t * ew + co + c
                        par = k % 2
                        pb = 64 * par
                        st = not started[par]
                        started[par] = True
                        nc.tensor.matmul(
                            gp[pb : pb + D + S, :],
                            lt[:, (co + c) * (D + S) : (co + c + 1) * (D + S)],
                            et[:, (co + c) * D : (co + c + 1) * D],
                            start=st,
                            stop=(par == 1 and k == cpp - 1),
                            tile_position=(0, pb),
                            skip_group_check=True,
                        )

            # tail chunk -> position 0 accumulator, closes its group
            etl = spool.tile([P, D], bf16, tag="etl")
            nc.sync.dma_start(out=etl[0:tail, :], in_=emb[u, main:ft, :])
            ltl = spool.tile([P, D + S], bf16, tag="ltl")
            nc.vector.tensor_mul(
                ltl[0:tail, 0:D],
                etl[0:tail, :],
                wtl[0:tail, :].broadcast_to([tail, D]),
            )
            nc.vector.tensor_copy(ltl[0:tail, D : D + S], wotl[0:tail, :])
            nc.tensor.matmul(
                gp[0 : D + S, :],
                ltl[0:tail, :],
                etl[0:tail, :],
                start=False,
                stop=True,
                tile_position=(0, 0),
                skip_group_check=True,
            )

            gsb = spool.tile([P, D], f32, tag="gsb")
            nc.scalar.activation(
                gsb[0 : D + S, :], gp[0 : D + S, :], mybir.ActivationFunctionType.Copy
            )
            nc.scalar.activation(
                gsb[64 : 64 + D + S, :],
                gp[64 : 64 + D + S, :],
                mybir.ActivationFunctionType.Copy,
            )
            nc.sync.dma_start(out=g_out[u, 0, :, :], in_=gsb[0 : D + S, :])
            nc.sync.dma_start(out=g_out[u, 1, :, :], in_=gsb[64 : 64 + D + S, :])

    nc.compile()
    return nc


EW2 = int(os.environ.get("DPCL_EW2", "240"))       # chunks per full tile (mult of 3)
OMEGA = int(os.environ.get("DPCL_OMEGA", "8"))     # wrep width (divides D)
# per-full-tile class chars, tiles in order (u0 t0..t4, u1 t0..t4):
#   c = SWDGE cast fp8->bf16 E + DVE mul
#   p = plain bf16 E (SP/ACT HWDGE) + DVE mul
#   g = raw fp8 E + GPSIMD mul (mixed-dtype matmul moving operand)
#   G = plain bf16 E + GPSIMD mul
PAT2 = os.environ.get("DPCL_PAT2", "ccpccccpcc")
WRENG = os.environ.get("DPCL_WRENG", "act")        # wrep engine: act|vec
ACC = os.environ.get("DPCL_ACC", "1") == "1"       # ride sums on accum_out
EBUFS2 = int(os.environ.get("DPCL_EBUFS2", "5"))
LBUFS2 = int(os.environ.get("DPCL_LBUFS2", "2"))


def _build_v2(nper, cpp, ew, tail, pat):
    """FWL-window Gram build with fully tiled prep.

    One 128-col LDWEIGHTS window per 3-chunk block (overlapping windows over
    the contiguous (c,e) L layout trigger Fast Weight Load), one N=120 matmul
    per block accumulating a block-diagonal [128,120] PSUM whose three 42x40
    diagonal blocks are summed on the host.  L columns per chunk:
    [w*E (40) | wo1 | w]; C0/b0 are recovered on the host as t - C1 / M - b1.
    All prep (w / mref-plane loads, argmax mask, wo1) happens in tile-sized
    slices inside the pipeline so there is no serial prologue."""
    import concourse.bacc as bacc
    import concourse.tile as tile
    from concourse import mybir

    f32 = mybir.dt.float32
    bf16 = mybir.dt.bfloat16
    fp8 = mybir.dt.float8e4
    ft = P * cpp + tail
    main = P * cpp
    D1 = D + S                       # 42 stationary cols per chunk
    NFULL_T = cpp // ew              # full tiles per utterance
    REM = cpp - NFULL_T * ew         # leftover chunks
    NTT = NFULL_T + (1 if REM else 0)
    assert ew % 3 == 0 and D % OMEGA == 0
    assert len(pat) == nper * NFULL_T, (pat, NFULL_T)

    nc = bacc.Bacc(
        "TRN2", target_bir_lowering=False, debug=False, num_devices=NCORES
    )
    emb8 = nc.declare_dram_parameter("emb8", [nper, ft, D], fp8, isOutput=False)
    emb16 = nc.declare_dram_parameter("emb16", [nper, ft, D], bf16, isOutput=False)
    # host-packed prep data: [u, P, 3, cpp+1] = (w | mref0 | mref1) rows per
    # partition, last column = tail values on partitions 0:tail
    prep_d = nc.declare_dram_parameter(
        "prep_d", [nper, P, 3, cpp + 1], bf16, isOutput=False
    )
    g_out = nc.declare_dram_parameter("g_out", [nper, P, 120], f32, isOutput=True)
    b_out = nc.declare_dram_parameter("b_out", [nper, P, S], f32, isOutput=True)

    with tile.TileContext(nc) as tc, ExitStack() as ctx:
        wpool = ctx.enter_context(tc.tile_pool(name="wpool", bufs=3))
        epool = ctx.enter_context(tc.tile_pool(name="epool", bufs=EBUFS2))
        lpool = ctx.enter_context(tc.tile_pool(name="lpool", bufs=LBUFS2))
        wrpool = ctx.enter_context(tc.tile_pool(name="wrpool", bufs=2))
        spool = ctx.enter_context(tc.tile_pool(name="spool", bufs=2))
        psum = ctx.enter_context(tc.tile_pool(name="psum", bufs=2, space="PSUM"))

        for u in range(nper):
            upat = pat[u * NFULL_T : (u + 1) * NFULL_T]
            e_main8 = emb8[u, 0:main, :].rearrange("(p c) d -> p c d", p=P)
            e_main16 = emb16[u, 0:main, :].rearrange("(p c) d -> p c d", p=P)

            # one packed prep load per utterance (w | mref0 | mref1 | tails)
            pk = wpool.tile([P, 3 * (cpp + 1)], bf16, tag="pk")
            pk3 = pk[:].rearrange("p (k c) -> p k c", k=3)
            nc.scalar.dma_start(out=pk3[:], in_=prep_d[u, :, :, :])
            # tail E cast early so the tail matmul never stalls the boundary
            etl = spool.tile([P, D], bf16, tag="etl")
            nc.gpsimd.dma_start(out=etl[0:tail, :], in_=emb8[u, main:ft, :])

            wo1p = spool.tile([P, 16], f32, tag="wo1p")   # per-tile sum wo1
            wp = spool.tile([P, 16], f32, tag="wp")       # per-tile sum w
            gp = psum.tile([P, 120], f32, tag="g")

            first = True
            tl = [(t * ew, ew, upat[t]) for t in range(NFULL_T)]
            if REM:
                tl.append((NFULL_T * ew, REM, "c"))
            gps_mm = []
            for ti, (co, cw, cls) in enumerate(tl):
                # -- per-tile prep from the packed load --
                w_sl = pk3[:, 0, co : co + cw]
                mr0 = pk3[:, 1, co : co + cw]
                mr1 = pk3[:, 2, co : co + cw]
                mask = wpool.tile([P, cw], bf16, tag="mask")
                nc.vector.tensor_tensor(
                    mask[:], mr1[:], mr0[:], mybir.AluOpType.is_gt
                )
                wo1w = wpool.tile([P, cw * 2], bf16, tag="wo1w")
                w2 = wo1w[:].rearrange("p (c s) -> p c s", s=2)
                if ACC:
                    nc.vector.tensor_tensor_reduce(
                        w2[:, :, 0], w_sl, mask[:], 1.0, 0.0,
                        mybir.AluOpType.mult, mybir.AluOpType.add,
                        wo1p[:, ti : ti + 1],
                    )
                else:
                    nc.vector.tensor_mul(w2[:, :, 0], w_sl, mask[:])
                    nc.vector.tensor_reduce(
                        wo1p[:, ti : ti + 1], w2[:, :, 0].unsqueeze(1),
                        mybir.AxisListType.X, mybir.AluOpType.add,
                    )
                nc.vector.tensor_copy(w2[:, :, 1], w_sl)
                nc.vector.tensor_reduce(
                    wp[:, ti : ti + 1], w_sl.unsqueeze(1),
                    mybir.AxisListType.X, mybir.AluOpType.add,
                )

                # -- E tile --
                et = epool.tile([P, cw * D], bf16, tag="e")
                e3 = et[:].rearrange("p (c d) -> p c d", d=D)
                if cls == "p":
                    nc.sync.dma_start(out=e3[:], in_=e_main16[:, co : co + cw, :])
                else:
                    nc.gpsimd.dma_start(out=e3[:], in_=e_main8[:, co : co + cw, :])

                # -- L tile --
                lt = lpool.tile([P, cw * D1 + 2], bf16, tag="l")
                l3 = lt[:, 0 : cw * D1].rearrange("p (c e) -> p c e", e=D1)
                nc.vector.memset(lt[:, cw * D1 : cw * D1 + 2], 0.0)
                wr = wrpool.tile([P, cw * OMEGA], bf16, tag="wr")
                wr3 = wr[:].rearrange("p (c d) -> p c d", d=OMEGA)
                wsl8 = w_sl.unsqueeze(2).broadcast_to([P, cw, OMEGA])
                nc.scalar.activation(
                    wr3[:], wsl8, mybir.ActivationFunctionType.Copy
                )
                if cls == "G":
                    wsl = w_sl.unsqueeze(2).broadcast_to([P, cw, D])
                    nc.gpsimd.tensor_mul(l3[:, :, 0:D], e3[:], wsl)
                else:
                    for j in range(D // OMEGA):
                        nc.vector.tensor_mul(
                            l3[:, :, j * OMEGA : (j + 1) * OMEGA],
                            e3[:, :, j * OMEGA : (j + 1) * OMEGA],
                            wr3[:],
                        )
                nc.vector.tensor_copy(l3[:, :, D : D + 2], w2[:, :, :])

                # -- FWL-window matmuls --
                nb = cw // 3
                mms = []
                for bb in range(nb):
                    mms.append((
                        lt[:, bb * 3 * D1 : bb * 3 * D1 + 128],
                        et[:, bb * 3 * D : (bb + 1) * 3 * D],
                        False,
                    ))
                for c in range(nb * 3, cw):
                    mms.append((
                        lt[:, c * D1 : (c + 1) * D1],
                        et[:, c * D : (c + 1) * D],
                        True,
                    ))
                if cls == "G":
                    gps_mm.extend(mms)
                    continue
                for lhsT, rhs, single in mms:
                    if single:
                        nc.tensor.matmul(
                            gp[0:D1, 0:D], lhsT, rhs,
                            start=False, stop=False,
                            tile_position=(0, 0), skip_group_check=True,
                        )
                    else:
                        nc.tensor.matmul(
                            gp[:, :], lhsT, rhs,
                            start=first, stop=False, skip_group_check=True,
                        )
                        first = False

            # deferred G-tile matmuls (L built by GpSimd long before)
            for lhsT, rhs, single in gps_mm:
                if single:
                    nc.tensor.matmul(
                        gp[0:D1, 0:D], lhsT, rhs,
                        start=False, stop=False,
                        tile_position=(0, 0), skip_group_check=True,
                    )
                else:
                    nc.tensor.matmul(
                        gp[:, :], lhsT, rhs,
                        start=False, stop=False, skip_group_check=True,
                    )

            # ---- tail chunk (inputs already on-chip via pk / early etl) ----
            wtl = pk3[:, 0, cpp : cpp + 1]
            masktl = spool.tile([P, 1], bf16, tag="masktl")
            nc.vector.tensor_tensor(
                masktl[0:tail, :],
                pk3[0:tail, 2, cpp : cpp + 1],
                pk3[0:tail, 1, cpp : cpp + 1],
                mybir.AluOpType.is_gt,
            )
            wo1tl = spool.tile([P, S], bf16, tag="wo1tl")
            nc.vector.tensor_mul(wo1tl[0:tail, 0:1], wtl[0:tail, :], masktl[0:tail, :])
            nc.vector.tensor_copy(wo1tl[0:tail, 1:2], wtl[0:tail, :])
            ltl = spool.tile([P, D1], bf16, tag="ltl")
            nc.vector.tensor_mul(
                ltl[0:tail, 0:D],
                etl[0:tail, :],
                wtl[0:tail, :].broadcast_to([tail, D]),
            )
            nc.vector.tensor_copy(ltl[0:tail, D : D + S], wo1tl[0:tail, :])
            nc.tensor.matmul(
                gp[0:D1, 0:D], ltl[0:tail, :], etl[0:tail, :],
                start=False, stop=True,
                tile_position=(0, 0), skip_group_check=True,
            )

            # ---- assemble b sums ----
            wored = spool.tile([P, S], f32, tag="wored")
            nc.vector.tensor_reduce(
                wored[:, 0:1], wo1p[:, 0:NTT].unsqueeze(1),
                mybir.AxisListType.X, mybir.AluOpType.add,
            )
            nc.vector.tensor_reduce(
                wored[:, 1:2], wp[:, 0:NTT].unsqueeze(1),
                mybir.AxisListType.X, mybir.AluOpType.add,
            )
            nc.vector.tensor_add(wored[0:tail, :], wored[0:tail, :], wo1tl[0:tail, :])
            nc.scalar.dma_start(out=b_out[u, :, :], in_=wored[:])
            gsb = spool.tile([P, 120], f32, tag="gsb")
            nc.scalar.activation(gsb[:], gp[:], mybir.ActivationFunctionType.Copy)
            nc.scalar.dma_start(out=g_out[u, :, :], in_=gsb[:])

    nc.compile()
    return nc


def _finish_host_v2(g_all, b_all):
    """g_all: [N, 128, 120] block-diagonal dumps, b_all: [N, P, 2] -> loss."""
    g = g_all.astype(np.float64)
    G = (
        g[:, 0:D1V, 0:D]
        + g[:, D1V : 2 * D1V, D : 2 * D]
        + g[:, 2 * D1V : 3 * D1V, 2 * D : 3 * D]
    )  # [N, 42, 40]
    b = b_all.astype(np.float64).sum(axis=1)  # [N, 2] = (b1, M)
    A = G[:, 0:D, :]
    C1 = G[:, D, :]
    t = G[:, D + 1, :]
    C0 = t - C1
    b1 = b[:, 0]
    M = b[:, 1]
    b0 = M - b1
    a2 = (A**2).sum(axis=(1, 2))
    c2 = (C0**2).sum(axis=1) + (C1**2).sum(axis=1)
    loss = (a2 + b0**2 + b1**2 - 2.0 * c2) / (M * M * T)
    return np.asarray(loss.mean(), dtype=np.float32)


D1V = D + S


def _get_program(key):
    if key not in _prog_cache:
        if key[-1] == "v2":
            _prog_cache[key] = _build_v2(*key[:-1], pat=PAT2)
        elif key[-1] == "perm":
            _prog_cache[key] = _build_perm(*key[:-1])
        elif key[-1] == "tile2":
            _prog_cache[key] = _build_tile2(
                *key[:-1], ng_pool=NG_POOL, prep_pool=PREP_POOL
            )
        else:
            _prog_cache[key] = _build_program(*key)
    return _prog_cache[key]


def _finish_host(g_all, b_all):
    """g_all: [N, 42, 40] (or [N, 2, 42, 40]), b_all: [N, P, 2] -> loss."""
    if g_all.ndim == 4:
        g_all = g_all.sum(axis=1, dtype=np.float64)
    g = g_all.astype(np.float64)
    b = b_all.astype(np.float64).sum(axis=1)  # [N, 2]
    a2 = (g[:, 0:D, :] ** 2).sum(axis=(1, 2))
    c2 = (g[:, D : D + S, :] ** 2).sum(axis=(1, 2))
    b2 = (b**2).sum(axis=1)
    m = b.sum(axis=1)
    loss = (a2 + b2 - 2.0 * c2) / (m * m * T)
    return np.asarray(loss.mean(), dtype=np.float32)


def _install_trace_shim():
    """Provide the antenv.axon_hooks module bass_utils expects for NTFF
    profiling under axon (this image's antenv lacks it)."""
    import sys as _sys
    import types

    if "antenv.axon_hooks" in _sys.modules:
        return
    try:
        from trn_agent_boot.trn_boot import _ntff_profile_via_ctypes

        hook = _ntff_profile_via_ctypes("/opt/axon/libaxon_pjrt.so")
    except Exception:
        hook = None
    mod = types.ModuleType("antenv.axon_hooks")
    mod.get_axon_ntff_profile_hook = lambda: hook
    mod.set_axon_ntff_profile_hook = lambda h: None
    _sys.modules["antenv.axon_hooks"] = mod


def kernel(embedding, magnitude_ref, magnitude_mix):
    from concourse.bass_utils import run_bass_kernel_spmd

    global LAST_EXEC_NS
    mref = np.ascontiguousarray(magnitude_ref, dtype=np.float32).reshape(N_FULL, FT, S)
    mm = np.ascontiguousarray(magnitude_mix, dtype=np.float32).reshape(N_FULL, FT)
    core_ids = list(range(NCORES))

    if MODE == "v2":
        import ml_dtypes

        emb32 = np.ascontiguousarray(embedding, dtype=np.float32)
        emb8 = emb32.astype(ml_dtypes.float8_e4m3fn)
        emb16 = emb32.astype(ml_dtypes.bfloat16)
        mm16 = mm.astype(ml_dtypes.bfloat16)
        mref16 = mref.astype(ml_dtypes.bfloat16)
        # packed prep tensor: [N, P, 3, CPP+1] = (w | mref0 | mref1) with the
        # tail (rows MAIN:FT) scattered into the last column, partitions 0:TAIL
        prep = np.zeros((N_FULL, P, 3, CPP + 1), dtype=ml_dtypes.bfloat16)
        prep[:, :, 0, :CPP] = mm16[:, :MAIN].reshape(N_FULL, P, CPP)
        prep[:, :, 1, :CPP] = mref16[:, :MAIN, 0].reshape(N_FULL, P, CPP)
        prep[:, :, 2, :CPP] = mref16[:, :MAIN, 1].reshape(N_FULL, P, CPP)
        prep[:, :TAIL, 0, CPP] = mm16[:, MAIN:]
        prep[:, :TAIL, 1, CPP] = mref16[:, MAIN:, 0]
        prep[:, :TAIL, 2, CPP] = mref16[:, MAIN:, 1]
        nc = _get_program((NPER, CPP, EW2, TAIL, "v2"))
        in_maps = [
            {
                "emb8": emb8[i * NPER : (i + 1) * NPER],
                "emb16": emb16[i * NPER : (i + 1) * NPER],
                "prep_d": prep[i * NPER : (i + 1) * NPER],
            }
            for i in core_ids
        ]
    elif MODE == "perm":
        import ml_dtypes

        emb32 = np.ascontiguousarray(embedding, dtype=np.float32)
        emb_p = (
            emb32[:, :MAIN, :]
            .reshape(N_FULL, P, CPP, D)
            .transpose(0, 1, 3, 2)
            .astype(ml_dtypes.bfloat16)
        )
        emb_t = emb32[:, MAIN:, :].astype(ml_dtypes.bfloat16)
        nc = _get_program((NPER, CPP, EW, CB, TAIL, "perm"))
        in_maps = [
            {
                "emb_p": emb_p[i * NPER : (i + 1) * NPER],
                "emb_t": emb_t[i * NPER : (i + 1) * NPER],
                "mm": mm[i * NPER : (i + 1) * NPER],
                "mref": mref[i * NPER : (i + 1) * NPER],
            }
            for i in core_ids
        ]
    elif MODE == "tile2":
        import ml_dtypes

        emb = np.ascontiguousarray(embedding).astype(ml_dtypes.bfloat16)
        mref = mref.astype(np.float16)
        mm = mm.astype(np.float16)
        nc = _get_program((NPER, CPP, EW, CB, TAIL, "tile2"))
        in_maps = [
            {
                "emb": emb[i * NPER : (i + 1) * NPER],
                "mm": mm[i * NPER : (i + 1) * NPER],
                "mref": mref[i * NPER : (i + 1) * NPER],
            }
            for i in core_ids
        ]
    else:
        if MODE == "bf16host":
            import ml_dtypes

            emb = np.ascontiguousarray(embedding).astype(ml_dtypes.bfloat16)
        else:
            emb = np.ascontiguousarray(embedding, dtype=np.float32)
        nc = _get_program((NPER, CPP, CB, NGROUPS, TAIL, MODE))
        in_maps = [
            {
                "emb": emb[i * NPER : (i + 1) * NPER],
                "mm": mm[i * NPER : (i + 1) * NPER],
                "mref": mref[i * NPER : (i + 1) * NPER],
            }
            for i in core_ids
        ]
    trace = os.environ.get("DPCL_TRACE", "0") == "1"
    if trace:
        _install_trace_shim()
    res = None
    for attempt in range(3):
        try:
            res = run_bass_kernel_spmd(nc, in_maps, core_ids, trace=trace)
            break
        except Exception:
            if attempt == 2:
                raise
    assert res is not None
    LAST_EXEC_NS = res.exec_time_ns

    g_all = np.concatenate([r["g_out"] for r in res.results], axis=0)
    b_all = np.concatenate([r["b_out"] for r in res.results], axis=0)
    if MODE == "v2":
        return _finish_host_v2(g_all, b_all)
    return _finish_host(g_all, b_all)

